# revision 1
# baseline (speedup 1.0000x reference)
"""Trainium2 Bass kernel for nn_MultiHeadAttention_32031866093611.

Sharding: pure data parallel — batch b -> NeuronCore b (B == n_cores == 8).
Weights replicated. No collectives.

Per-core program (batch b, S=1024, D=1024, H=16, DK=64), all matmuls fp32r:

  inputs (per core): xT = x[b].T [D, S], Wq/Wk/Wv/Wo [D, D] (as stored),
                     bq/bk/bv/bo [1, D], masks (host-built from prefix[b]).

  qT[c]   = (Wq[:, c*128:+128]).T @ xT + bq       -> [128 d', 1024 s]   (8 chunks)
  kT[c]   = same with Wk                          -> [128 d', 1024 s]
  v[sc]   = (xT[:, sc*128:+128]).T @ Wv + bv      -> [128 s, 16, 64+1]  (ones col)
  per head h (c=h//2, r=h%2*64):
    for kc in 0..7:
      sT[kc] = kT[c][r:r+64, kc*128:+128].T @ qT[c][r:r+64, :]   # [128 k, 1024 q]
      sT[kc] += diag/column additive masks (DVE, on cols >= kc*128)
      eT[kc] = exp(sT[kc])                                        # ACT, psum->sbuf
      outT  += v[kc][:, h, :].T @ eT[kc]       # [65, 1024]: row 64 = softmax denom
    attnT[c][r:r+64, :] = outT[0:64, :] * bcast(1/outT[64, :])
  out[sc] = (attnT[.][:, sc*128:+128]).T @ Wo + bo  -> [128 s, 1024 d] -> DRAM

The mask allowed(q,k) = (q < prefix) | (k >= q) decomposes in the transposed
[k, q] tile grid as: blocks kc > qc fully allowed (untouched); everything at or
below the diagonal (cols >= kc*128) gets one multiplicative 0/1 u8 mask applied
to the exp output on DVE (exp(s)*m == exp(s + additive mask)).

Schedule: flat (h, kc) stream with PV matmuls lagging scores/exp by 3 tiles
(in-order PE never waits on a just-issued exp); o_proj chunk k (which only
needs heads 2k, 2k+1 after the interleave) is emitted two heads after head
2k+1 retires, inside the ACT-bound attention phase; ~20 warm-up matmuls keep
the PE HAM clock-gate hot while the first x/Wq DMAs land.
"""

import numpy as np

import concourse.bass as bass
import concourse.mybir as mybir
import concourse.tile as tile
from concourse import bacc
from concourse.bass_utils import run_bass_kernel_spmd

B, S, D, H = 8, 1024, 1024, 16
DK = D // H  # 64
P = 128
NCHUNK = S // P  # 8
NCORES = 8
F32R = mybir.dt.float32r
F32 = mybir.dt.float32
EXP = mybir.ActivationFunctionType.Exp
NEG = -1.0e30
HALF = 512  # fp32 moving-operand max
MSK_OFF = [0]
for _kc in range(1, 8):
    MSK_OFF.append(MSK_OFF[-1] + S - (_kc - 1) * P)

_CACHED = {}


def build_nc(repeats=1):
    nc = bacc.Bacc("TRN2", target_bir_lowering=False, debug=False, num_devices=NCORES)

    xt_d = nc.dram_tensor("xt", [D, S], F32R, kind="ExternalInput").ap()
    wq_d = nc.dram_tensor("wq", [D, D], F32R, kind="ExternalInput").ap()
    wk_d = nc.dram_tensor("wk", [D, D], F32R, kind="ExternalInput").ap()
    wv_d = nc.dram_tensor("wv", [D, D], F32R, kind="ExternalInput").ap()
    wo_d = nc.dram_tensor("wo", [D, D], F32R, kind="ExternalInput").ap()
    bqk_d = nc.dram_tensor("bqk", [P, 2 * NCHUNK], F32, kind="ExternalInput").ap()
    ones_d = nc.dram_tensor("ones2d", [P, P], F32R, kind="ExternalInput").ap()
    bv_d = nc.dram_tensor("bv", [P, D], F32, kind="ExternalInput").ap()
    bo_d = nc.dram_tensor("bo", [P, D], F32, kind="ExternalInput").ap()
    msk_d = nc.dram_tensor("mask8", [P, 4608], mybir.dt.uint8, kind="ExternalInput").ap()
    out_d = nc.dram_tensor("out", [S, D], F32, kind="ExternalOutput").ap()

    with tile.TileContext(nc) as tc:
        with (
            tc.tile_pool(name="w", bufs=18) as wpool,
            tc.tile_pool(name="big", bufs=2) as bigpool,
            tc.tile_pool(name="qk", bufs=8) as qkpool,
            tc.tile_pool(name="v", bufs=8) as vpool,
            tc.tile_pool(name="cst", bufs=1) as cstpool,
            tc.tile_pool(name="exp", bufs=5) as exppool,
            tc.tile_pool(name="rcp", bufs=1) as rcppool,
            tc.tile_pool(name="rbc", bufs=1) as rbcpool,
            tc.tile_pool(name="osb", bufs=1) as osbpool,
            tc.tile_pool(name="pp", bufs=2, space="PSUM") as pp,
            tc.tile_pool(name="po", bufs=2, space="PSUM") as po,
        ):
            for _rep in range(repeats):
                # ---- x chunks + Wq strips interleaved (fast PE start), cst after ----
                ones2d = cstpool.tile([P, P], F32R, tag="ones2d")
                nc.sync.dma_start(ones2d[:], ones_d[:])
                ones = ones2d[0:1, :]
                xtq = [
                    bigpool.tile([P, 4, S], F32R, tag="big", name=f"xtq_{g}")
                    for g in range(2)
                ]
                def whalf(nm, w_dram, hf):
                    """8 half-strips [128, 512] of W columns [hf*512, (hf+1)*512)."""
                    ts = [
                        wpool.tile([P, HALF], F32R, tag="w", name=f"{nm}{hf}_{dc}")
                        for dc in range(NCHUNK)
                    ]
                    sl = slice(hf * HALF, (hf + 1) * HALF)
                    for dc in range(NCHUNK):
                        nc.sync.dma_start(ts[dc][:], w_dram[dc * P : (dc + 1) * P, sl])
                    return ts

                for dc in range(NCHUNK):
                    nc.sync.dma_start(
                        xtq[dc // 4][:, dc % 4, 0:HALF],
                        xt_d[dc * P : (dc + 1) * P, 0:HALF],
                    )
                qh0 = whalf("wq", wq_d, 0)
                for dc in range(NCHUNK):
                    nc.sync.dma_start(
                        xtq[dc // 4][:, dc % 4, HALF:S],
                        xt_d[dc * P : (dc + 1) * P, HALF:S],
                    )
                kh0 = whalf("wk", wk_d, 0)
                xt = [xtq[dc // 4][:, dc % 4, :] for dc in range(NCHUNK)]

                # PE warm-up: ~20 throwaway matmuls on the first-arriving tiny
                # tile keep the HAM clock-gate busy while x/Wq stream in.
                wps = pp.tile([P, S], F32, tag="pp", name="warmup_ps")
                for wi in range(18):
                    nc.tensor.matmul(
                        wps[:, 0:P], ones2d[:], ones2d[:], start=True, stop=True
                    )
                bqk = cstpool.tile([P, 2 * NCHUNK], F32, tag="bqk")
                nc.sync.dma_start(bqk[:], bqk_d[:])
                msk = cstpool.tile([P, 4608], mybir.dt.uint8, tag="msk")
                nc.sync.dma_start(msk[:], msk_d[:])
                bias = {}
                # bv (v-proj) and bo (o_proj) lifetimes don't overlap: share slot
                bias["bv"] = cstpool.tile([P, D], F32, tag="bvbo", name="bv_bc")
                nc.sync.dma_start(bias["bv"][:], bv_d[:])

                # ---- helper: dense [d', s] projection (qT / kT) ----
                def proj_half(whalf_tiles, chalf, bcol0, out_tag):
                    """qT/kT chunks chalf*4 .. chalf*4+3 from one W column half."""
                    outs = []
                    for cp in range(2):
                        cs = (chalf * 4 + 2 * cp, chalf * 4 + 2 * cp + 1)
                        pss = {
                            c: pp.tile([P, S], F32, tag="pp", name=f"ps_{out_tag}_{c}")
                            for c in cs
                        }
                        for j in range(2):
                            sl = slice(j * HALF, (j + 1) * HALF)
                            for c in cs:
                                lc = (c % 4) * P
                                for dc in range(NCHUNK):
                                    nc.tensor.matmul(
                                        pss[c][:, sl],
                                        whalf_tiles[dc][:, lc : lc + P],
                                        xt[dc][:, sl],
                                        start=(dc == 0),
                                        stop=(dc == NCHUNK - 1),
                                    )
                        for c in cs:
                            o = qkpool.tile(
                                [P, S], F32R, tag=out_tag, name=f"{out_tag}_{c}"
                            )
                            nc.vector.tensor_add(
                                o[:],
                                pss[c][:],
                                bqk[:, bcol0 + c : bcol0 + c + 1].to_broadcast((P, S)),
                            )
                            outs.append(o)
                    return outs

                with nc.named_scope("qk_proj"):
                    qT = proj_half(qh0, 0, 0, "qT")
                    qh1 = whalf("wq", wq_d, 1)
                    kT = proj_half(kh0, 0, NCHUNK, "kT")
                    kh1 = whalf("wk", wk_d, 1)
                    qT += proj_half(qh1, 1, 0, "qT")
                    kT += proj_half(kh1, 1, NCHUNK, "kT")

                # ---- v projection: [s, 16, 65] with ones column ----
                with nc.named_scope("v_proj"):
                    vh = [whalf("wv", wv_d, 0), whalf("wv", wv_d, 1)]
                    vtiles = []
                    for sc in range(NCHUNK):
                        ps = pp.tile([P, S], F32, tag="pp")
                        for j in range(2):
                            sl = slice(j * HALF, (j + 1) * HALF)
                            for dc in range(NCHUNK):
                                nc.tensor.matmul(
                                    ps[:, sl],
                                    xt[dc][:, sc * P : (sc + 1) * P],
                                    vh[j][dc][:],
                                    start=(dc == 0),
                                    stop=(dc == NCHUNK - 1),
                                )
                        vt = vpool.tile([P, H, DK + 1], F32R, tag="v")
                        nc.vector.tensor_add(
                            vt[:, :, 0:DK],
                            ps[:].rearrange("p (h d) -> p h d", h=H),
                            bias["bv"][:].rearrange("p (h d) -> p h d", h=H),
                        )
                        nc.vector.tensor_copy(
                            vt[:, :, DK : DK + 1], ones2d[:, 0:1].to_broadcast((P, H, 1))
                        )
                        vtiles.append(vt)

                # ---- attention heads ----
                bias["bo"] = cstpool.tile([P, D], F32, tag="bvbo", name="bo_bc")
                nc.sync.dma_start(bias["bo"][:], bo_d[:])
                attn = [None, None]

                # Wo strips prefetched before the head loop (slots free up as
                # Wq/Wk strips retire); o_proj chunk sc only needs heads 2sc,2sc+1.
                oh = [whalf("wo", wo_d, 0), whalf("wo", wo_d, 1)]

                def emit_scores_exp(h, kc):
                    """scores on PE, exp on ACT, multiplicative 0/1 masks on DVE."""
                    c, r = h // 2, (h % 2) * DK
                    pss = pp.tile([P, S], F32, tag="pp", name=f"pss_{h}_{kc}")
                    lhs = kT[c][r : r + DK, kc * P : (kc + 1) * P]
                    for j in range(2):
                        sl = slice(j * HALF, (j + 1) * HALF)
                        nc.tensor.matmul(
                            pss[:, sl],
                            lhs,
                            qT[c][r : r + DK, sl],
                            start=True,
                            stop=True,
                        )
                    et = exppool.tile([P, S], F32R, tag="exp", name=f"et_{h}_{kc}")
                    nc.scalar.activation(et[:], pss[:], EXP)
                    # one 0/1 mask mult over cols [kc*128, 1024): diag pattern on
                    # the diagonal block, column mask below the diagonal
                    w = S - kc * P
                    off = MSK_OFF[kc]
                    nc.vector.tensor_mul(
                        et[:, kc * P : S], et[:, kc * P : S], msk[:, off : off + w]
                    )
                    return et

                def emit_pv(h, kc, pso, et):
                    for j in range(2):
                        sl = slice(j * HALF, (j + 1) * HALF)
                        nc.tensor.matmul(
                            pso[0 : DK + 1, sl],
                            vtiles[kc][:, h, :],
                            et[:, sl],
                            start=(kc == 0),
                            stop=(kc == NCHUNK - 1),
                        )

                def emit_norm(h, pso):
                    rcp = rcppool.tile([1, S], F32, tag="rcp", name=f"rcp_{h}")
                    nc.vector.reciprocal(rcp[:], pso[DK : DK + 1, :])
                    rbc = rbcpool.tile([DK, S], F32, tag="rbc", name=f"rbc_{h}")
                    nc.gpsimd.partition_broadcast(rbc[:], rcp[:])
                    # attn[g][e*64+d, cc, h*64+u] = O_h[u*16 + 2*(4g+cc) + e, d]/denom
                    src = pso[0:DK, :].rearrange("d (u j) -> d j u", j=16)
                    rbs = rbc[:].rearrange("d (u j) -> d j u", j=16)
                    for g in range(2):
                        if attn[g] is None:
                            attn[g] = bigpool.tile(
                                [P, 4, S], F32R, tag="big", name=f"attnq_{g}"
                            )
                        for e in range(2):
                            jsl = slice(8 * g + e, 8 * (g + 1), 2)
                            nc.vector.tensor_mul(
                                attn[g][e * DK : (e + 1) * DK, :, h * DK : (h + 1) * DK],
                                src[:, jsl, :],
                                rbs[:, jsl, :],
                            )

                def emit_oproj(sc):
                    ps = po.tile([P, S], F32, tag="po", name=f"psf_{sc}")
                    for j in range(2):
                        sl = slice(j * HALF, (j + 1) * HALF)
                        for cc in range(NCHUNK):
                            nc.tensor.matmul(
                                ps[:, sl],
                                attn[cc // 4][:, cc % 4, sc * P : (sc + 1) * P],
                                oh[j][cc][:],
                                start=(cc == 0),
                                stop=(cc == NCHUNK - 1),
                            )
                    ot = osbpool.tile([P, S], F32, tag="osb", name=f"ot_{sc}")
                    nc.vector.tensor_add(ot[:], ps[:], bias["bo"][:])
                    nc.sync.dma_start(out_d[sc * P : (sc + 1) * P, :], ot[:])

                # Flat (h, kc) stream, PV lagging scores/exp by one tile so the
                # in-order PE never waits on a just-issued exp. After the last
                # PV of a head, the accumulator is copied to SBUF immediately to
                # free its PSUM bank; the norm chain reads the copy. o_proj
                # chunk k (needs heads 2k,2k+1 only) is emitted two heads later.
                from collections import deque
                pend = deque()
                pso_cur = None

                def pop_pv():
                    ph, pkc, ppso, pet = pend.popleft()
                    emit_pv(ph, pkc, ppso, pet)
                    if pkc == NCHUNK - 1:
                        emit_norm(ph, ppso)
                        if ph % 2 == 1 and ph >= 3:
                            emit_oproj((ph - 3) // 2)

                for h in range(H):
                    pso_cur = po.tile([P, S], F32, tag="po", name=f"pso_{h}")
                    for kc in range(NCHUNK):
                        et = emit_scores_exp(h, kc)
                        if len(pend) >= 4:
                            pop_pv()
                        pend.append((h, kc, pso_cur, et))
                while len(pend) > 1:
                    pop_pv()
                # last PV of head 15: slot o_proj(6) in front of the norm chain
                # so the PE stays busy while recip/bcast run on DVE/Pool.
                ph, pkc, ppso, pet = pend.popleft()
                emit_pv(ph, pkc, ppso, pet)
                emit_oproj(NCHUNK - 2)
                emit_norm(ph, ppso)
                emit_oproj(NCHUNK - 1)

    nc.compile()
    return nc


def _host_masks(prefix_b: int):
    """Combined multiplicative 0/1 mask, u8, applied to exp output.

    For scores-T tile kc (cols q in [kc*128, 1024)): element (i, q) keeps
    exp iff allowed(q, k=kc*128+i) = (q < prefix) or (k >= q).
    Segment kc occupies msk[:, off_kc : off_kc + (1024 - kc*128)].
    """
    i = np.arange(P)[:, None]
    segs = []
    for kc in range(NCHUNK):
        q = np.arange(kc * P, S)[None, :]
        k = kc * P + i
        allowed = (q < prefix_b) | (k >= q)
        segs.append(allowed.astype(np.uint8))
    return np.concatenate(segs, axis=1)


def kernel(x, prefix, Wq, bq, Wk, bk, Wv, bv, Wo, bo, _trace=False):
    x = np.asarray(x, dtype=np.float32)
    prefix = np.asarray(prefix)
    Wq, Wk, Wv, Wo = (np.ascontiguousarray(np.asarray(w, np.float32)) for w in (Wq, Wk, Wv, Wo))
    bv, bo = (
        np.broadcast_to(np.asarray(v, np.float32).reshape(1, D), (P, D)).copy()
        for v in (bv, bo)
    )
    bqk = np.stack(
        [np.asarray(bq, np.float32).reshape(NCHUNK, P), np.asarray(bk, np.float32).reshape(NCHUNK, P)], axis=0
    ).reshape(2 * NCHUNK, P).T.copy()  # [128, 16]: cols 0-7 = bq chunks, 8-15 = bk

    ones2d = np.ones((P, P), dtype=np.float32)
    if "nc" not in _CACHED:
        _CACHED["nc"] = build_nc()
    nc = _CACHED["nc"]

    in_maps = []
    for b in range(B):
        mask8 = _host_masks(int(prefix[b]))
        in_maps.append(
            {
                "xt": np.ascontiguousarray(x[b].T),
                "wq": Wq, "wk": Wk, "wv": Wv, "wo": Wo,
                "bqk": bqk, "bv": bv, "bo": bo, "ones2d": ones2d,
                "mask8": mask8,
            }
        )

    res = run_bass_kernel_spmd(nc, in_maps, core_ids=list(range(NCORES)), trace=_trace)
    out = np.stack([res.results[b]["out"] for b in range(B)], axis=0)
    if _trace:
        return out, res
    return out



# revision 4
# speedup vs baseline: 1.0729x; 1.0729x over previous
"""Trainium2 Bass kernel for nn_MultiHeadAttention_32031866093611.

Sharding: pure data parallel — batch b -> NeuronCore b (B == n_cores == 8).
Weights replicated. No collectives.

Per-core program (batch b, S=1024, D=1024, H=16, DK=64), matmuls fp32r except
PV (bf16 probs x bf16 v):

  inputs (per core): xT = x[b].T [D, S], Wq/Wk/Wv/Wo [D, D] (as stored),
                     bq/bk/bv/bo, masks (host-built from prefix[b], bf16).

  qT[c]   = (Wq[:, c*128:+128]).T @ xT + bq       -> [128 d', 1024 s]   (8 chunks)
  kT[c]   = same with Wk                          -> [128 d', 1024 s]
  v[sc]   = (xT[:, sc*128:+128]).T @ Wv + bv      -> [128 s, 16, 128] bf16
            (cols 64:128 memset to 1.0: PV then yields the softmax denominator
             replicated across 64 partitions for free — no partition broadcast)
  per head h (c=h//2, r=h%2*64):
    for kc in 0..7:
      sT[kc] = kT[c][r:r+64, kc*128:+128].T @ qT[c][r:r+64, :]   # [128 k, 1024 q]
      eT[kc] = exp(sT[kc])  (ACT, psum->sbuf, bf16 out)
      eT[kc] *= 0/1 bf16 mask on cols >= kc*128 (DVE, 2x bf16 mode)
      outT  += v[kc][:, h, :].T @ eT[kc]   # [128, 1024]: rows 64:128 = denom
    attnT[c][r:r+64, :] = outT[0:64, :] * bcast(1/outT[64:128, :])
  out[sc] = (attnT[.][:, sc*128:+128]).T @ Wo + bo  (bias+DMA per 512-col half)

Schedule: flat (h, kc) stream with PV matmuls lagging scores/exp; o_proj chunk
k (heads 2k, 2k+1) emitted two heads later; warm-up matmuls on a memset tile
keep the PE clock ramp hot from t~0; x is DMAed in 4 batched strips and the
tiny bqk bias lands right after the first weight strips so the first psum
drain never stalls.
"""

import numpy as np
import ml_dtypes

import concourse.bass as bass
import concourse.mybir as mybir
import concourse.tile as tile
from concourse import bacc
from concourse.bass_utils import run_bass_kernel_spmd

B, S, D, H = 8, 1024, 1024, 16
DK = D // H  # 64
P = 128
NCHUNK = S // P  # 8
NCORES = 8
F32R = mybir.dt.float32r
F32 = mybir.dt.float32
BF16 = mybir.dt.bfloat16
EXP = mybir.ActivationFunctionType.Exp
HALF = 512  # fp32 moving-operand max
MSK_OFF = [0]
for _kc in range(1, 8):
    MSK_OFF.append(MSK_OFF[-1] + S - (_kc - 1) * P)

_CACHED = {}


def build_nc(repeats=1):
    nc = bacc.Bacc("TRN2", target_bir_lowering=False, debug=False, num_devices=NCORES)

    xt_d = nc.dram_tensor("xt", [D, S], F32R, kind="ExternalInput").ap()
    wq_d = nc.dram_tensor("wq", [D, D], F32R, kind="ExternalInput").ap()
    wk_d = nc.dram_tensor("wk", [D, D], F32R, kind="ExternalInput").ap()
    wv_d = nc.dram_tensor("wv", [D, D], F32R, kind="ExternalInput").ap()
    wo_d = nc.dram_tensor("wo", [D, D], F32R, kind="ExternalInput").ap()
    bqk_d = nc.dram_tensor("bqk", [P, 2 * NCHUNK], F32, kind="ExternalInput").ap()
    bv_d = nc.dram_tensor("bv", [P, D], F32, kind="ExternalInput").ap()
    bo_d = nc.dram_tensor("bo", [P, D], F32, kind="ExternalInput").ap()
    msk_d = nc.dram_tensor("mask8", [P, 4608], BF16, kind="ExternalInput").ap()
    out_d = nc.dram_tensor("out", [S, D], F32, kind="ExternalOutput").ap()

    with tile.TileContext(nc) as tc:
        with (
            tc.tile_pool(name="w", bufs=18) as wpool,
            tc.tile_pool(name="big", bufs=2) as bigpool,
            tc.tile_pool(name="qk", bufs=8) as qkpool,
            tc.tile_pool(name="v", bufs=8) as vpool,
            tc.tile_pool(name="cst", bufs=1) as cstpool,
            tc.tile_pool(name="exp", bufs=5) as exppool,
            tc.tile_pool(name="rcp", bufs=1) as rcppool,
            tc.tile_pool(name="osb", bufs=2) as osbpool,
            tc.tile_pool(name="pp", bufs=2, space="PSUM") as pp,
            tc.tile_pool(name="po", bufs=2, space="PSUM") as po,
        ):
            for _rep in range(repeats):
                # ---- warm-up on a memset tile: PE ramping from t~0 ----
                wup = cstpool.tile([P, P], BF16, tag="wup")
                nc.vector.memset(wup[:], 0.0)
                wps = pp.tile([P, S], F32, tag="pp", name="warmup_ps")
                for wi in range(20):
                    nc.tensor.matmul(
                        wps[:, 0:P], wup[:], wup[:], start=True, stop=True
                    )

                # ---- batched x strips + Wq strips (fast PE start) ----
                xtq = [
                    bigpool.tile([P, 4, S], F32R, tag="big", name=f"xtq_{g}")
                    for g in range(2)
                ]

                def x_half(g, hf):
                    """One DMA for x chunks 4g..4g+3, column half hf."""
                    sl = slice(hf * HALF, (hf + 1) * HALF)
                    nc.sync.dma_start(
                        xtq[g][:, :, sl],
                        xt_d[g * HALF : (g + 1) * HALF, sl].rearrange(
                            "(c p) q -> p c q", p=P
                        ),
                    )

                def whalf(nm, w_dram, hf):
                    """8 half-strips [128, 512] of W columns [hf*512, (hf+1)*512)."""
                    ts = [
                        wpool.tile([P, HALF], F32R, tag="w", name=f"{nm}{hf}_{dc}")
                        for dc in range(NCHUNK)
                    ]
                    sl = slice(hf * HALF, (hf + 1) * HALF)
                    for dc in range(NCHUNK):
                        nc.sync.dma_start(ts[dc][:], w_dram[dc * P : (dc + 1) * P, sl])
                    return ts

                x_half(0, 0)
                x_half(1, 0)
                qh0 = whalf("wq", wq_d, 0)
                bqk = cstpool.tile([P, 2 * NCHUNK], F32, tag="bqk")
                nc.sync.dma_start(bqk[:], bqk_d[:])
                x_half(0, 1)
                x_half(1, 1)
                kh0 = whalf("wk", wk_d, 0)
                bias = {}
                # bv (v-proj) and bo (o_proj) lifetimes don't overlap: share slot
                bias["bv"] = cstpool.tile([P, D], F32, tag="bvbo", name="bv_bc")
                nc.sync.dma_start(bias["bv"][:], bv_d[:])
                msk = cstpool.tile([P, 4608], BF16, tag="msk")
                nc.sync.dma_start(msk[:], msk_d[:])
                xt = [xtq[dc // 4][:, dc % 4, :] for dc in range(NCHUNK)]

                # ---- helper: dense [d', s] projection (qT / kT) ----
                def proj_half(whalf_tiles, chalf, bcol0, out_tag):
                    """qT/kT chunks chalf*4 .. chalf*4+3 from one W column half."""
                    outs = []
                    for cp in range(2):
                        cs = (chalf * 4 + 2 * cp, chalf * 4 + 2 * cp + 1)
                        pss = {
                            c: pp.tile([P, S], F32, tag="pp", name=f"ps_{out_tag}_{c}")
                            for c in cs
                        }
                        for j in range(2):
                            sl = slice(j * HALF, (j + 1) * HALF)
                            for c in cs:
                                lc = (c % 4) * P
                                for dc in range(NCHUNK):
                                    nc.tensor.matmul(
                                        pss[c][:, sl],
                                        whalf_tiles[dc][:, lc : lc + P],
                                        xt[dc][:, sl],
                                        start=(dc == 0),
                                        stop=(dc == NCHUNK - 1),
                                    )
                        for c in cs:
                            o = qkpool.tile(
                                [P, S], F32R, tag=out_tag, name=f"{out_tag}_{c}"
                            )
                            nc.vector.tensor_add(
                                o[:],
                                pss[c][:],
                                bqk[:, bcol0 + c : bcol0 + c + 1].to_broadcast((P, S)),
                            )
                            outs.append(o)
                    return outs

                with nc.named_scope("qk_proj"):
                    qT = proj_half(qh0, 0, 0, "qT")
                    qh1 = whalf("wq", wq_d, 1)
                    kT = proj_half(kh0, 0, NCHUNK, "kT")
                    kh1 = whalf("wk", wk_d, 1)
                    qT += proj_half(qh1, 1, 0, "qT")
                    kT += proj_half(kh1, 1, NCHUNK, "kT")

                # ---- v projection: [s, 16, 128] bf16, cols 64:128 = 1.0 ----
                with nc.named_scope("v_proj"):
                    vh = [whalf("wv", wv_d, 0), whalf("wv", wv_d, 1)]
                    vtiles = []
                    for sc in range(NCHUNK):
                        vt = vpool.tile([P, H, 2 * DK], BF16, tag="v")
                        nc.vector.memset(vt[:, :, DK : 2 * DK], 1.0)
                        ps = pp.tile([P, S], F32, tag="pp")
                        for j in range(2):
                            sl = slice(j * HALF, (j + 1) * HALF)
                            for dc in range(NCHUNK):
                                nc.tensor.matmul(
                                    ps[:, sl],
                                    xt[dc][:, sc * P : (sc + 1) * P],
                                    vh[j][dc][:],
                                    start=(dc == 0),
                                    stop=(dc == NCHUNK - 1),
                                )
                        nc.vector.tensor_add(
                            vt[:, :, 0:DK],
                            ps[:].rearrange("p (h d) -> p h d", h=H),
                            bias["bv"][:].rearrange("p (h d) -> p h d", h=H),
                        )
                        vtiles.append(vt)

                # ---- attention heads ----
                bias["bo"] = cstpool.tile([P, D], F32, tag="bvbo", name="bo_bc")
                nc.sync.dma_start(bias["bo"][:], bo_d[:])
                attn = [None, None]

                # Wo strips prefetched before the head loop (slots free up as
                # Wq/Wk strips retire); o_proj chunk sc only needs heads 2sc,2sc+1.
                oh = [whalf("wo", wo_d, 0), whalf("wo", wo_d, 1)]

                def emit_scores_exp(h, kc):
                    """scores on PE, exp on ACT (bf16 out), 0/1 bf16 masks on DVE."""
                    c, r = h // 2, (h % 2) * DK
                    pss = pp.tile([P, S], F32, tag="pp", name=f"pss_{h}_{kc}")
                    lhs = kT[c][r : r + DK, kc * P : (kc + 1) * P]
                    for j in range(2):
                        sl = slice(j * HALF, (j + 1) * HALF)
                        nc.tensor.matmul(
                            pss[:, sl],
                            lhs,
                            qT[c][r : r + DK, sl],
                            start=True,
                            stop=True,
                        )
                    et = exppool.tile([P, S], BF16, tag="exp", name=f"et_{h}_{kc}")
                    nc.scalar.activation(et[:], pss[:], EXP)
                    # one 0/1 mask mult over cols [kc*128, 1024): diag pattern on
                    # the diagonal block, column mask below the diagonal
                    w = S - kc * P
                    off = MSK_OFF[kc]
                    nc.vector.tensor_mul(
                        et[:, kc * P : S], et[:, kc * P : S], msk[:, off : off + w]
                    )
                    return et

                def emit_pv(h, kc, pso, et):
                    for j in range(2):
                        sl = slice(j * HALF, (j + 1) * HALF)
                        nc.tensor.matmul(
                            pso[:, sl],
                            vtiles[kc][:, h, :],
                            et[:, sl],
                            start=(kc == 0),
                            stop=(kc == NCHUNK - 1),
                        )

                def emit_norm(h, pso):
                    rcp = rcppool.tile([DK, S], F32, tag="rcp", name=f"rcp_{h}")
                    nc.vector.reciprocal(rcp[:], pso[DK : 2 * DK, :])
                    # attn[g][e*64+d, cc, h*64+u] = O_h[u*16 + 2*(4g+cc) + e, d]/denom
                    src = pso[0:DK, :].rearrange("d (u j) -> d j u", j=16)
                    rbs = rcp[:].rearrange("d (u j) -> d j u", j=16)
                    for g in range(2):
                        if attn[g] is None:
                            attn[g] = bigpool.tile(
                                [P, 4, S], F32R, tag="big", name=f"attnq_{g}"
                            )
                        for e in range(2):
                            jsl = slice(8 * g + e, 8 * (g + 1), 2)
                            nc.vector.tensor_mul(
                                attn[g][e * DK : (e + 1) * DK, :, h * DK : (h + 1) * DK],
                                src[:, jsl, :],
                                rbs[:, jsl, :],
                            )

                def emit_oproj(sc):
                    ps = po.tile([P, S], F32, tag="po", name=f"psf_{sc}")
                    for j in range(2):
                        sl = slice(j * HALF, (j + 1) * HALF)
                        for cc in range(NCHUNK):
                            nc.tensor.matmul(
                                ps[:, sl],
                                attn[cc // 4][:, cc % 4, sc * P : (sc + 1) * P],
                                oh[j][cc][:],
                                start=(cc == 0),
                                stop=(cc == NCHUNK - 1),
                            )
                        ot = osbpool.tile(
                            [P, HALF], F32, tag="osb", name=f"ot_{sc}_{j}"
                        )
                        nc.vector.tensor_add(ot[:], ps[:, sl], bias["bo"][:, sl])
                        nc.sync.dma_start(out_d[sc * P : (sc + 1) * P, sl], ot[:])

                # Flat (h, kc) stream, PV lagging scores/exp so the in-order PE
                # never waits on a just-issued exp. After the last PV of a head
                # the norm chain reads the psum; o_proj chunk k (needs heads
                # 2k,2k+1 only) is emitted two heads later.
                from collections import deque
                pend = deque()
                pso_cur = None

                def pop_pv():
                    ph, pkc, ppso, pet = pend.popleft()
                    emit_pv(ph, pkc, ppso, pet)
                    if pkc == NCHUNK - 1:
                        emit_norm(ph, ppso)
                        if ph % 2 == 1 and ph >= 3:
                            emit_oproj((ph - 3) // 2)

                for h in range(H):
                    pso_cur = po.tile([P, S], F32, tag="po", name=f"pso_{h}")
                    for kc in range(NCHUNK):
                        et = emit_scores_exp(h, kc)
                        if len(pend) >= 4:
                            pop_pv()
                        pend.append((h, kc, pso_cur, et))
                while len(pend) > 1:
                    pop_pv()
                # last PV of head 15: slot o_proj(6) in front of the norm chain
                # so the PE stays busy while the reciprocal runs on DVE.
                ph, pkc, ppso, pet = pend.popleft()
                emit_pv(ph, pkc, ppso, pet)
                emit_oproj(NCHUNK - 2)
                emit_norm(ph, ppso)
                emit_oproj(NCHUNK - 1)

    nc.compile()
    return nc


def _host_masks(prefix_b: int):
    """Combined multiplicative 0/1 mask, bf16, applied to exp output.

    For scores-T tile kc (cols q in [kc*128, 1024)): element (i, q) keeps
    exp iff allowed(q, k=kc*128+i) = (q < prefix) or (k >= q).
    Segment kc occupies msk[:, off_kc : off_kc + (1024 - kc*128)].
    """
    i = np.arange(P)[:, None]
    segs = []
    for kc in range(NCHUNK):
        q = np.arange(kc * P, S)[None, :]
        k = kc * P + i
        allowed = (q < prefix_b) | (k >= q)
        segs.append(allowed.astype(ml_dtypes.bfloat16))
    return np.concatenate(segs, axis=1)


def kernel(x, prefix, Wq, bq, Wk, bk, Wv, bv, Wo, bo, _trace=False):
    x = np.asarray(x, dtype=np.float32)
    prefix = np.asarray(prefix)
    Wq, Wk, Wv, Wo = (np.ascontiguousarray(np.asarray(w, np.float32)) for w in (Wq, Wk, Wv, Wo))
    bv, bo = (
        np.broadcast_to(np.asarray(v, np.float32).reshape(1, D), (P, D)).copy()
        for v in (bv, bo)
    )
    bqk = np.stack(
        [np.asarray(bq, np.float32).reshape(NCHUNK, P), np.asarray(bk, np.float32).reshape(NCHUNK, P)], axis=0
    ).reshape(2 * NCHUNK, P).T.copy()  # [128, 16]: cols 0-7 = bq chunks, 8-15 = bk

    if "nc" not in _CACHED:
        _CACHED["nc"] = build_nc()
    nc = _CACHED["nc"]

    in_maps = []
    for b in range(B):
        mask8 = _host_masks(int(prefix[b]))
        in_maps.append(
            {
                "xt": np.ascontiguousarray(x[b].T),
                "wq": Wq, "wk": Wk, "wv": Wv, "wo": Wo,
                "bqk": bqk, "bv": bv, "bo": bo,
                "mask8": mask8,
            }
        )

    res = run_bass_kernel_spmd(nc, in_maps, core_ids=list(range(NCORES)), trace=_trace)
    out = np.stack([res.results[b]["out"] for b in range(B)], axis=0)
    if _trace:
        return out, res
    return out


# revision 19
# speedup vs baseline: 1.1047x; 1.0297x over previous
"""Trainium2 Bass kernel for nn_MultiHeadAttention_32031866093611.

Sharding: pure data parallel — batch b -> NeuronCore b (B == n_cores == 8).
Weights replicated. No collectives.

Per-core program (batch b, S=1024, D=1024, H=16, DK=64):

  qT[c]   = (Wq[:, c*128:+128]).T @ xT + bq  -> [128 d', 1024 s] bf16  (fp32r mm)
  kT[c]   = same with Wk                     -> [128 d', 1024 s] bf16
  v[sc]   = (xT[:, sc*128:+128]).T @ Wv + bv -> [128 s, 16, 128] bf16
            (cols 64:128 memset to 1.0 so PV emits the softmax denominator
             replicated across 64 partitions for free — no partition bcast)
  per head h (c=h//2, r=h%2*64), kc DESCENDING 7..0:
      sT[kc] = kT[c][r:r+64, kc*128:+128].T @ qT[c][r:r+64, 0:hi]  # [128 k, hi q]
      eT[kc] = exp(sT[kc])  (ACT, psum->sbuf, bf16)
      eT[kc] *= 0/1 bf16 mask on cols [kc*128, hi)  (DVE 2x bf16 mode)
      outT  += v[kc][:, h, :].T @ eT[kc][:, 0:hi]  # [128, 1024]: 64:128 = denom
    attnT[c][r:r+64, :] = outT[0:64, :] * (1/outT[64:128, :])  -> bf16
  out[sc] = (attnT[.][:, sc*128:+128]).T @ Wo16 + bo  (bias + DMA per half)

hi = hi_kc = max(TRIM, (kc+1)*128) clipped to 1024, TRIM = max(prefix): cols
q >= TRIM strictly below the diagonal are masked for EVERY core, so their
scores/exp/mask/PV work is skipped; descending kc makes the per-tile PV
column ranges nest, keeping psum accumulation start/stop valid. The program
is rebuilt (cached) per distinct TRIM.

Schedule: flat (h, kc) stream with PV lagging scores/exp; o_proj chunk k
(heads 2k, 2k+1) emitted two heads later, chunks 5+6 held back to cover the
final norm window; warm-up matmuls on a memset tile keep the PE clock ramp
hot from t~0; startup DMAs ordered so the first projection matmul's operands
(x chunk 0 + first Wq strip) land first.
"""

import numpy as np
import ml_dtypes

import concourse.bass as bass
import concourse.mybir as mybir
import concourse.tile as tile
from concourse import bacc
from concourse.bass_utils import run_bass_kernel_spmd

B, S, D, H = 8, 1024, 1024, 16
DK = D // H  # 64
P = 128
NCHUNK = S // P  # 8
NCORES = 8
F32R = mybir.dt.float32r
F32 = mybir.dt.float32
BF16 = mybir.dt.bfloat16
EXP = mybir.ActivationFunctionType.Exp
HALF = 512  # fp32 moving-operand / psum-bank max

_CACHED = {}


def _tile_hi(trim):
    """Per-kc live column bound: cols q >= max(trim, (kc+1)*128) are dead."""
    return [min(max(trim, (kc + 1) * P), S) for kc in range(NCHUNK)]


def build_nc(trim=S, repeats=1, parts=frozenset({"scores", "exp", "mask", "pv"})):
    hi = _tile_hi(trim)
    hi_s = hi if "scores" in parts else [S] * NCHUNK
    hi_e = hi if "exp" in parts else [S] * NCHUNK
    hi_p = hi if "pv" in parts else [S] * NCHUNK
    assert all(p <= e <= s for p, e, s in zip(hi_p, hi_e, hi_s))
    msk_off = {}
    off = 0
    for kc in range(NCHUNK):
        msk_off[kc] = off
        off += hi[kc] - kc * P
    msk_len = off

    nc = bacc.Bacc("TRN2", target_bir_lowering=False, debug=False, num_devices=NCORES)

    xt_d = nc.dram_tensor("xt", [D, S], F32R, kind="ExternalInput").ap()
    wq_d = nc.dram_tensor("wq", [D, D], F32R, kind="ExternalInput").ap()
    wk_d = nc.dram_tensor("wk", [D, D], F32R, kind="ExternalInput").ap()
    wv_d = nc.dram_tensor("wv", [D, D], F32R, kind="ExternalInput").ap()
    wo_d = nc.dram_tensor("wo", [D, D], BF16, kind="ExternalInput").ap()
    bqk_d = nc.dram_tensor("bqk", [P, 2 * NCHUNK], F32, kind="ExternalInput").ap()
    bv_d = nc.dram_tensor("bv", [P, D], F32, kind="ExternalInput").ap()
    bo_d = nc.dram_tensor("bo", [P, D], F32, kind="ExternalInput").ap()
    msk_d = nc.dram_tensor("mask8", [P, msk_len], BF16, kind="ExternalInput").ap()
    out_d = nc.dram_tensor("out", [S, D], F32, kind="ExternalOutput").ap()

    with tile.TileContext(nc) as tc:
        with (
            tc.tile_pool(name="w", bufs=18) as wpool,
            tc.tile_pool(name="big", bufs=2) as bigpool,
            tc.tile_pool(name="qk", bufs=8) as qkpool,
            tc.tile_pool(name="v", bufs=8) as vpool,
            tc.tile_pool(name="cst", bufs=1) as cstpool,
            tc.tile_pool(name="exp", bufs=5) as exppool,
            tc.tile_pool(name="rcp", bufs=1) as rcppool,
            tc.tile_pool(name="osb", bufs=3) as osbpool,
            tc.tile_pool(name="pp", bufs=2, space="PSUM") as pp,
            tc.tile_pool(name="po", bufs=2, space="PSUM") as po,
        ):
            for _rep in range(repeats):
                # ---- warm-up on a memset tile: PE ramping from t~0 ----
                wup = cstpool.tile([P, P], BF16, tag="wup")
                nc.vector.memset(wup[:], 0.0)
                wps = pp.tile([P, S], F32, tag="pp", name="warmup_ps")
                for wi in range(34):
                    nc.tensor.matmul(
                        wps[:, 0:P], wup[:], wup[:], start=True, stop=True
                    )

                # ---- startup DMAs, ordered by first use ----
                xtq = [
                    bigpool.tile([P, 4, S], F32R, tag="big", name=f"xtq_{g}")
                    for g in range(2)
                ]

                def x_part(g, hf, c0, c1):
                    """One DMA for x chunks 4g+c0..4g+c1-1, column half hf."""
                    sl = slice(hf * HALF, (hf + 1) * HALF)
                    nc.sync.dma_start(
                        xtq[g][:, c0:c1, sl],
                        xt_d[
                            g * HALF + c0 * P : g * HALF + c1 * P, sl
                        ].rearrange("(c p) q -> p c q", p=P),
                    )

                def whalf(nm, w_dram, hf, lo=0, hi_=NCHUNK, dt=F32R):
                    """Half-strips [128, 512] of W columns [hf*512, (hf+1)*512)."""
                    ts = []
                    sl = slice(hf * HALF, (hf + 1) * HALF)
                    for dc in range(lo, hi_):
                        t = wpool.tile([P, HALF], dt, tag="w", name=f"{nm}{hf}_{dc}")
                        nc.sync.dma_start(t[:], w_dram[dc * P : (dc + 1) * P, sl])
                        ts.append(t)
                    return ts

                x_part(0, 0, 0, 1)
                qh0 = whalf("wq", wq_d, 0, 0, 1)
                x_part(0, 0, 1, 4)
                qh0 += whalf("wq", wq_d, 0, 1, 4)
                x_part(1, 0, 0, 4)
                qh0 += whalf("wq", wq_d, 0, 4, 8)
                bqk = cstpool.tile([P, 2 * NCHUNK], F32, tag="bqk")
                nc.sync.dma_start(bqk[:], bqk_d[:])
                x_part(0, 1, 0, 4)
                x_part(1, 1, 0, 4)
                kh0 = whalf("wk", wk_d, 0)
                bias = {}
                # bv (v-proj) and bo (o_proj) lifetimes don't overlap: share slot
                bias["bv"] = cstpool.tile([P, D], F32, tag="bvbo", name="bv_bc")
                nc.sync.dma_start(bias["bv"][:], bv_d[:])
                msk = cstpool.tile([P, msk_len], BF16, tag="msk")
                nc.sync.dma_start(msk[:], msk_d[:])
                xt = [xtq[dc // 4][:, dc % 4, :] for dc in range(NCHUNK)]

                # ---- helper: dense [d', s] projection (qT / kT), bf16 out ----
                def proj_half(whalf_tiles, chalf, bcol0, out_tag):
                    """qT/kT chunks chalf*4 .. chalf*4+3 from one W column half."""
                    outs = []
                    for cp in range(2):
                        cs = (chalf * 4 + 2 * cp, chalf * 4 + 2 * cp + 1)
                        pss = {
                            c: pp.tile([P, S], F32, tag="pp", name=f"ps_{out_tag}_{c}")
                            for c in cs
                        }
                        for j in range(2):
                            sl = slice(j * HALF, (j + 1) * HALF)
                            for c in cs:
                                lc = (c % 4) * P
                                for dc in range(NCHUNK):
                                    nc.tensor.matmul(
                                        pss[c][:, sl],
                                        whalf_tiles[dc][:, lc : lc + P],
                                        xt[dc][:, sl],
                                        start=(dc == 0),
                                        stop=(dc == NCHUNK - 1),
                                    )
                        for c in cs:
                            o = qkpool.tile(
                                [P, S], BF16, tag=out_tag, name=f"{out_tag}_{c}"
                            )
                            nc.vector.tensor_add(
                                o[:],
                                pss[c][:],
                                bqk[:, bcol0 + c : bcol0 + c + 1].to_broadcast((P, S)),
                            )
                            outs.append(o)
                    return outs

                with nc.named_scope("qk_proj"):
                    qT = proj_half(qh0, 0, 0, "qT")
                    qh1 = whalf("wq", wq_d, 1)
                    kT = proj_half(kh0, 0, NCHUNK, "kT")
                    kh1 = whalf("wk", wk_d, 1)
                    qT += proj_half(qh1, 1, 0, "qT")
                    kT += proj_half(kh1, 1, NCHUNK, "kT")

                # ---- v projection: [s, 16, 128] bf16, cols 64:128 = 1.0 ----
                with nc.named_scope("v_proj"):
                    vh = [whalf("wv", wv_d, 0), whalf("wv", wv_d, 1)]
                    vtiles = []
                    for sc in range(NCHUNK):
                        vt = vpool.tile([P, H, 2 * DK], BF16, tag="v")
                        nc.vector.memset(vt[:, :, DK : 2 * DK], 1.0)
                        ps = pp.tile([P, S], F32, tag="pp")
                        for j in range(2):
                            sl = slice(j * HALF, (j + 1) * HALF)
                            for dc in range(NCHUNK):
                                nc.tensor.matmul(
                                    ps[:, sl],
                                    xt[dc][:, sc * P : (sc + 1) * P],
                                    vh[j][dc][:],
                                    start=(dc == 0),
                                    stop=(dc == NCHUNK - 1),
                                )
                        nc.vector.tensor_add(
                            vt[:, :, 0:DK],
                            ps[:].rearrange("p (h d) -> p h d", h=H),
                            bias["bv"][:].rearrange("p (h d) -> p h d", h=H),
                        )
                        vtiles.append(vt)

                # ---- attention heads ----
                bias["bo"] = cstpool.tile([P, D], F32, tag="bvbo", name="bo_bc")
                nc.sync.dma_start(bias["bo"][:], bo_d[:])
                attn = [None, None]

                # Wo bf16 strips prefetched before the head loop.
                oh = [whalf("wo", wo_d, 0, dt=BF16), whalf("wo", wo_d, 1, dt=BF16)]

                def emit_scores_exp(h, kc):
                    """scores on PE, exp on ACT (bf16 out), 0/1 bf16 masks on DVE."""
                    c, r = h // 2, (h % 2) * DK
                    pss = pp.tile([P, S], F32, tag="pp", name=f"pss_{h}_{kc}")
                    lhs = kT[c][r : r + DK, kc * P : (kc + 1) * P]
                    for lo in range(0, hi_s[kc], HALF):
                        sl = slice(lo, min(lo + HALF, hi_s[kc]))
                        nc.tensor.matmul(
                            pss[:, sl],
                            lhs,
                            qT[c][r : r + DK, sl],
                            start=True,
                            stop=True,
                        )
                    et = exppool.tile([P, S], BF16, tag="exp", name=f"et_{h}_{kc}")
                    nc.scalar.activation(
                        et[:, 0 : hi_e[kc]], pss[:, 0 : hi_e[kc]], EXP
                    )
                    # one 0/1 mask mult over cols [kc*128, hi): diag pattern on
                    # the diagonal block, column mask below the diagonal
                    w = hi[kc] - kc * P
                    off = msk_off[kc]
                    nc.vector.tensor_mul(
                        et[:, kc * P : hi[kc]],
                        et[:, kc * P : hi[kc]],
                        msk[:, off : off + w],
                    )
                    return et

                def emit_pv(h, kc, pso, et):
                    # kc descending: ranges nest. PSUM start=True zeroing is
                    # BANK-granular, so the first writer (kc=7, whose range is
                    # always full) must cover each 512-col bank in ONE
                    # start=True matmul; later kc's accumulate prefixes of the
                    # bank with start=False. stop=True goes on each bank's
                    # last writer (the smallest kc that still reaches it).
                    first = kc == NCHUNK - 1
                    for b0 in range(0, hi_p[kc], HALF):
                        hi2 = min(b0 + HALF, S) if first else min(b0 + HALF, hi_p[kc])
                        last = all(hi_p[k2] <= b0 for k2 in range(kc))
                        nc.tensor.matmul(
                            pso[:, b0:hi2],
                            vtiles[kc][:, h, :],
                            et[:, b0:hi2],
                            start=first,
                            stop=last,
                        )

                def emit_norm(h, pso):
                    rcp = rcppool.tile([DK, S], F32, tag="rcp", name=f"rcp_{h}")
                    nc.vector.reciprocal(rcp[:], pso[DK : 2 * DK, :])
                    # attn[g][e*64+d, cc, h*64+u] = O_h[u*16 + 2*(4g+cc) + e, d]/denom
                    src = pso[0:DK, :].rearrange("d (u j) -> d j u", j=16)
                    rbs = rcp[:].rearrange("d (u j) -> d j u", j=16)
                    for g in range(2):
                        if attn[g] is None:
                            attn[g] = bigpool.tile(
                                [P, 4, S], BF16, tag="big", name=f"attnq_{g}"
                            )
                        for e in range(2):
                            jsl = slice(8 * g + e, 8 * (g + 1), 2)
                            nc.vector.tensor_mul(
                                attn[g][e * DK : (e + 1) * DK, :, h * DK : (h + 1) * DK],
                                src[:, jsl, :],
                                rbs[:, jsl, :],
                            )

                def emit_oproj(sc):
                    ps = po.tile([P, S], F32, tag="po", name=f"psf_{sc}")
                    for j in range(2):
                        sl = slice(j * HALF, (j + 1) * HALF)
                        for cc in range(NCHUNK):
                            nc.tensor.matmul(
                                ps[:, sl],
                                attn[cc // 4][:, cc % 4, sc * P : (sc + 1) * P],
                                oh[j][cc][:],
                                start=(cc == 0),
                                stop=(cc == NCHUNK - 1),
                            )
                        ot = osbpool.tile(
                            [P, HALF], F32, tag="osb", name=f"ot_{sc}_{j}"
                        )
                        nc.vector.tensor_add(ot[:], ps[:, sl], bias["bo"][:, sl])
                        nc.sync.dma_start(out_d[sc * P : (sc + 1) * P, sl], ot[:])

                # Flat (h, kc-descending) stream, PV lagging scores/exp so the
                # in-order PE never waits on a just-issued exp. o_proj chunk k
                # (needs heads 2k,2k+1 only) runs two heads later; chunks 5+6
                # held for the final norm window so the PE stays busy at the
                # tail.
                from collections import deque
                pend = deque()
                pso_cur = None

                def pop_pv():
                    ph, pkc, ppso, pet = pend.popleft()
                    emit_pv(ph, pkc, ppso, pet)
                    if pkc == 0:
                        emit_norm(ph, ppso)
                        if ph % 2 == 1 and 3 <= ph <= 11:
                            emit_oproj((ph - 3) // 2)

                for h in range(H):
                    pso_cur = po.tile([P, S], F32, tag="po", name=f"pso_{h}")
                    for kc in range(NCHUNK - 1, -1, -1):
                        et = emit_scores_exp(h, kc)
                        if len(pend) >= 4:
                            pop_pv()
                        pend.append((h, kc, pso_cur, et))
                while len(pend) > 1:
                    pop_pv()
                # last PV of head 15: norm emitted first so DVE starts at once;
                # o_proj chunks 5+6 keep the PE busy under the norm chain.
                ph, pkc, ppso, pet = pend.popleft()
                emit_pv(ph, pkc, ppso, pet)
                emit_norm(ph, ppso)
                emit_oproj(NCHUNK - 3)
                emit_oproj(NCHUNK - 2)
                emit_oproj(NCHUNK - 1)

    nc.compile()
    return nc


def _host_masks(prefix_b: int, trim: int):
    """Combined multiplicative 0/1 mask, bf16, applied to exp output.

    For scores-T tile kc (cols q in [kc*128, hi_kc)): element (i, q) keeps
    exp iff allowed(q, k=kc*128+i) = (q < prefix) or (k >= q).
    """
    hi = _tile_hi(trim)
    i = np.arange(P)[:, None]
    segs = []
    for kc in range(NCHUNK):
        q = np.arange(kc * P, hi[kc])[None, :]
        k = kc * P + i
        allowed = (q < prefix_b) | (k >= q)
        segs.append(allowed.astype(ml_dtypes.bfloat16))
    return np.concatenate(segs, axis=1)


def kernel(x, prefix, Wq, bq, Wk, bk, Wv, bv, Wo, bo, _trace=False):
    x = np.asarray(x, dtype=np.float32)
    prefix = np.asarray(prefix)
    Wq, Wk, Wv = (
        np.ascontiguousarray(np.asarray(w, np.float32)) for w in (Wq, Wk, Wv)
    )
    Wo16 = np.ascontiguousarray(
        np.asarray(Wo, np.float32).astype(ml_dtypes.bfloat16)
    )
    bv, bo = (
        np.broadcast_to(np.asarray(v, np.float32).reshape(1, D), (P, D)).copy()
        for v in (bv, bo)
    )
    bqk = np.stack(
        [np.asarray(bq, np.float32).reshape(NCHUNK, P), np.asarray(bk, np.float32).reshape(NCHUNK, P)], axis=0
    ).reshape(2 * NCHUNK, P).T.copy()  # [128, 16]: cols 0-7 = bq chunks, 8-15 = bk

    # cols q >= max(prefix) below the diagonal are masked on every core:
    # specialize (and cache) the program on that bound.
    trim = int(prefix.max())
    if _CACHED.get("trim") != trim:
        _CACHED["nc"] = build_nc(trim=trim)
        _CACHED["trim"] = trim
    nc = _CACHED["nc"]

    in_maps = []
    for b in range(B):
        mask8 = _host_masks(int(prefix[b]), trim)
        in_maps.append(
            {
                "xt": np.ascontiguousarray(x[b].T),
                "wq": Wq, "wk": Wk, "wv": Wv, "wo": Wo16,
                "bqk": bqk, "bv": bv, "bo": bo,
                "mask8": mask8,
            }
        )

    res = run_bass_kernel_spmd(nc, in_maps, core_ids=list(range(NCORES)), trace=_trace)
    out = np.stack([res.results[b]["out"] for b in range(B)], axis=0)
    if _trace:
        return out, res
    return out
